# revision 17
# baseline (speedup 1.0000x reference)
"""Multi-head attention TRN2 kernel, 8-core (batch x head-block) sharded.

Problem (hardcoded): x[2,2048,1024] f32, Wq/Wk/Wv[1024,1024], Wo[1024,1024],
16 heads, dh=64. Reference computes softmax(Q K^T)/sqrt(1024) @ V @ Wo with the
division AFTER softmax (folded here into Wo as a host-side 1/32 scale).

Sharding: core c handles batch b=c//4 and head block hb=c%4 (4 heads = 256 dims:
Wq/Wk/Wv column slice, Wo row slice). Each core emits a partial Y[2048,1024]
in fp16; host sums the 4 partials per batch in fp32.

v5: bf16 datapath, 64x128 row-tiled QK^T (two heads concurrently on PE tiles
T0/T8 since dh=64), single-op approx reciprocal, fp16 output partials, and a
software-pipelined schedule: only K + a quarter of Q are projected up front;
the attention exp stream (the ACT-engine bottleneck, ~1.1us per [128,1024]
tile) starts ~18us in and the remaining Q/V projections + deferred PV
matmuls fill the tensor engine's slack under it. This also keeps PE activity
high so the HAM clock gate stays at 2.4GHz (an idle phase transition
re-throttles it to 1.2GHz and phase 2 then never re-warms).
"""

from contextlib import ExitStack

import numpy as np
import ml_dtypes

import concourse.tile as tile
from concourse import bacc, mybir
from concourse.bass_utils import run_bass_kernel_spmd

N_CORES = 8
B = 2
S = 2048          # tokens per batch (= per core)
D = 1024          # model dim
DH = 64           # head dim
HPC = 4           # heads per core
NP = HPC // 2     # head pairs per core (pair p = heads 2p, 2p+1)
DL = HPC * DH     # 256 local output dims per core
NK = D // 128     # 8 k-strips for QKV contraction
NT = S // 128     # 16 token strips
VW = 128          # V block: cols 0:64 = V dims, cols 64:128 = ones (denoms)

F32 = mybir.dt.float32
F16 = mybir.dt.float16
BF16 = mybir.dt.bfloat16
EXP = mybir.ActivationFunctionType.Exp
MULT = mybir.AluOpType.mult


def build_nc():
    nc = bacc.Bacc("TRN2", target_bir_lowering=False, debug=False)
    # weights are host-swizzled into their SBUF layouts so each loads as one
    # contiguous full-line DMA
    xT = nc.declare_dram_parameter("xT", [D, S], BF16, isOutput=False)
    Wq = nc.declare_dram_parameter("Wq", [128, NK * NP * 128], BF16,
                                   isOutput=False)
    Wk = nc.declare_dram_parameter("Wk", [128, NK * NP * 128], BF16,
                                   isOutput=False)
    Wv = nc.declare_dram_parameter("Wv", [128, NK * DL], BF16, isOutput=False)
    Wo = nc.declare_dram_parameter("Wo", [128, NP * D], BF16, isOutput=False)
    Yp = nc.declare_dram_parameter("Yp", [S, D], F16, isOutput=True)

    with tile.TileContext(nc) as tc, ExitStack() as st:
        singles = st.enter_context(tc.tile_pool(name="singles", bufs=1))
        wq_sb = singles.tile([128, NK * NP * 128], BF16)
        wk_sb = singles.tile([128, NK * NP * 128], BF16)
        wv_sb = singles.tile([128, NK * DL], BF16)
        wo_sb = singles.tile([128, NP * D], BF16)
        # qt/kt: pair p block = cols [p*S, (p+1)*S); rows 0:64 = head 2p
        # dims, rows 64:128 = head 2p+1 dims (feeds PE tiles T0/T8).
        qt_sb = singles.tile([128, NP * S], BF16)
        kt_sb = singles.tile([128, NP * S], BF16)
        ot_sb = singles.tile([128, NP * S], BF16)
        vaug_sb = singles.tile([128, HPC * NT * VW], BF16)

        # ones columns of vaug (denominator rows of pO)
        for blk in range(HPC * NT):
            nc.gpsimd.memset(vaug_sb[:, blk * VW + DH:(blk + 1) * VW], 1.0)

        # phase-2 pools opened first: pools must close in LIFO order, and the
        # projection pools (pst) close mid-phase2 to hand their PSUM banks
        # to the pO accumulators; all phase-2 pools close before phase 3
        ph2 = ExitStack()
        pSp = ph2.enter_context(tc.tile_pool(name="pS", bufs=2, space="PSUM"))
        expp = ph2.enter_context(tc.tile_pool(name="expp", bufs=40))
        normp = ph2.enter_context(tc.tile_pool(name="normp", bufs=2))

        # ---- projection-phase pools (closed mid-phase2, before pO opens) --
        pst = ExitStack()
        xkp = pst.enter_context(tc.tile_pool(name="xkp", bufs=4))
        pp1 = pst.enter_context(tc.tile_pool(name="pp1", bufs=2, space="PSUM"))
        ppv = pst.enter_context(tc.tile_pool(name="ppv", bufs=2, space="PSUM"))

        xcs = []

        def load_chunk(c):
            xc = xkp.tile([128, NK * 512], BF16, name="xk")
            nc.sync.dma_start(
                out=xc[:].rearrange("p (k s) -> p k s", k=NK),
                in_=xT[:, c * 512:(c + 1) * 512]
                .rearrange("(k p) s -> p k s", p=128),
            )
            xcs.append(xc)

        load_chunk(0)
        nc.sync.dma_start(out=wk_sb[:], in_=Wk[:, :])
        load_chunk(1)
        load_chunk(2)
        load_chunk(3)
        nc.sync.dma_start(out=wq_sb[:], in_=Wq[:, :])
        nc.sync.dma_start(out=wv_sb[:], in_=Wv[:, :])
        nc.sync.dma_start(out=wo_sb[:], in_=Wo[:, :])

        lead_engines = (nc.scalar, nc.vector)
        eci = [0]

        def copy_any(dst, src, lead):
            # lead-in copies alternate ACT/DVE; during phase 2 ACT is the
            # exp bottleneck so everything goes to DVE
            eng = lead_engines[eci[0] % 2] if lead else nc.vector
            eci[0] += 1
            if eng is nc.scalar:
                eng.copy(out=dst, in_=src)
            else:
                eng.tensor_copy(out=dst, in_=src)

        ps_box = {}

        def proj_qk_half(w_sb, o_sb, g, c, part, lead=False):
            # one projection output tile split into two 4-matmul emissions so
            # pass-(0,0) interleaving stays finer than the exp cadence
            if part == 0:
                ps_box[(g, c, id(w_sb))] = pp1.tile([128, 512], F32,
                                                    name="ps_qk")
            ps = ps_box[(g, c, id(w_sb))]
            for k in range(4 * part, 4 * part + 4):
                cb = (k * NP + g) * 128
                nc.tensor.matmul(
                    ps[:],
                    w_sb[:, cb:cb + 128],
                    xcs[c][:, k * 512:(k + 1) * 512],
                    start=(k == 0),
                    stop=(k == NK - 1),
                )
            if part == 1:
                del ps_box[(g, c, id(w_sb))]
                copy_any(o_sb[:, g * S + c * 512:g * S + (c + 1) * 512],
                         ps[:], lead)

        def proj_qk(w_sb, o_sb, g, c, lead=False):
            proj_qk_half(w_sb, o_sb, g, c, 0, lead)
            proj_qk_half(w_sb, o_sb, g, c, 1, lead)

        def proj_v(j, lead=False):
            c, t = divmod(j, 4)
            pv = ppv.tile([128, DL], F32, name="pv")
            for k in range(NK):
                nc.tensor.matmul(
                    pv[:],
                    xcs[c][:, k * 512 + t * 128:k * 512 + (t + 1) * 128],
                    wv_sb[:, k * DL:(k + 1) * DL],
                    start=(k == 0),
                    stop=(k == NK - 1),
                )
            for h in range(HPC):
                vb = (h * NT + j) * VW
                copy_any(vaug_sb[:, vb:vb + DH],
                         pv[:, h * DH:(h + 1) * DH], lead)

        # ---- lead-in: K pair 0 + the first Q quarter (pass (0,0) inputs) --
        proj_qk(wk_sb, kt_sb, 0, 0, lead=True)
        proj_qk(wk_sb, kt_sb, 0, 1, lead=True)
        proj_qk(wk_sb, kt_sb, 0, 2, lead=True)
        proj_qk(wk_sb, kt_sb, 0, 3, lead=True)
        proj_qk(wq_sb, qt_sb, 0, 0, lead=True)
        proj_qk(wq_sb, qt_sb, 0, 1, lead=True)

        # remaining projection work, paced into pass-(0,0) tensor slack;
        # (cost_us, emit_fn). K pair 1 and Qg1 c0/c1 are needed at pass
        # (1,0) start, V at the first deferred-PV drain, Q c2/c3 only by
        # the sh=1 passes.
        proj_units = [(1.7, lambda c=c: proj_qk(wk_sb, kt_sb, 1, c))
                      for c in range(4)]
        proj_units += [(1.7, lambda g=1, c=0: proj_qk(wq_sb, qt_sb, g, c)),
                       (1.7, lambda g=1, c=1: proj_qk(wq_sb, qt_sb, g, c))]
        proj_units += [(0.85, lambda j=j: proj_v(j)) for j in range(NT)]
        proj_units += [(1.7, lambda g=g, c=c: proj_qk(wq_sb, qt_sb, g, c))
                       for g, c in ((0, 2), (0, 3), (1, 2), (1, 3))]

        # ---- phase 2: attention ------------------------------------------
        pOp_box = [None]

        pO_tiles = {}
        pv_fifo = []

        def emit_qk_exp(p, sh, j, half):
            r0 = half * DH
            pS = pSp.tile([128, 1024], F32, name="pS")
            for sc in range(2):
                nc.tensor.matmul(
                    pS[:, sc * 512:(sc + 1) * 512],
                    kt_sb[r0:r0 + DH,
                          p * S + j * 128:p * S + (j + 1) * 128],
                    qt_sb[r0:r0 + DH,
                          p * S + sh * 1024 + sc * 512:
                          p * S + sh * 1024 + (sc + 1) * 512],
                )
            expst = expp.tile([128, 1024], BF16, name="expst")
            nc.scalar.activation(expst[:], pS[:], EXP)
            pv_fifo.append((expst, p, sh, j, half))

        def drain_pv(n):
            for _ in range(min(n, len(pv_fifo))):
                expst, p, sh, j, half = pv_fifo.pop(0)
                key = (p, sh, half)
                if key not in pO_tiles:
                    pO_tiles[key] = pOp_box[0].tile([VW, 1024], F32,
                                                    name="pO")
                pO = pO_tiles[key]
                h = 2 * p + half
                vb = (h * NT + j) * VW
                for sc in range(2):
                    nc.tensor.matmul(
                        pO[:, sc * 512:(sc + 1) * 512],
                        vaug_sb[:, vb:vb + VW],
                        expst[:, sc * 512:(sc + 1) * 512],
                        start=(j == 0),
                        stop=(j == NT - 1),
                        skip_group_check=True,
                    )
                if j == NT - 1:
                    emit_norm(p, sh, half)

        def emit_norm(p, sh, half):
            # pO rows 64:128 all hold the softmax denominator (vaug cols
            # 64:128 are ones): one approx reciprocal (custom-DVE ops drop
            # the input partition offset, so run it over all 128 partitions
            # and use rows 64:128), then scale the V rows.
            pO = pO_tiles.pop((p, sh, half))
            r = half * DH
            rb = normp.tile([128, 1024], F32, name="rb")
            nc.vector.reciprocal_approx_fast(rb[:], pO[:])
            nc.vector.tensor_tensor(
                out=ot_sb[r:r + DH,
                          p * S + sh * 1024:p * S + (sh + 1) * 1024],
                in0=pO[0:DH, :],
                in1=rb[DH:128, :],
                op=MULT,
            )

        # pass (0,0): QK+exp only (PV deferred until V is projected); the
        # remaining projections are paced into the slack by a credit budget
        # (~0.9us of PE slack per strip-half at the ~1.1us exp cadence).
        credits = 0.0
        for j in range(NT):
            for half in range(2):
                emit_qk_exp(0, 0, j, half)
                credits += 1.0
                while proj_units and credits >= proj_units[0][0]:
                    cost, fn = proj_units.pop(0)
                    credits -= cost
                    fn()
        # spill any leftover projection work, then free its PSUM pools so
        # the pO accumulators can take those banks
        for _, fn in proj_units:
            fn()
        pst.close()
        pOp_box[0] = ph2.enter_context(
            tc.tile_pool(name="pO", bufs=2, space="PSUM"))

        for pi, (p, sh) in enumerate(((1, 0), (0, 1), (1, 1))):
            last = pi == 2
            for j in range(NT):
                for half in range(2):
                    emit_qk_exp(p, sh, j, half)
                    # skip drains for the first strips of a pass: the first
                    # PV of a pass waits for a pO slot (freed by the prev
                    # pass's norm) and would head-of-line-block later QKs;
                    # drain harder near the end of the last pass to shrink
                    # the post-exp tail
                    if j >= 3:
                        drain_pv(3 if (last and j >= 12) else 2)
        drain_pv(len(pv_fifo))

        # ---- phase 3: output projection Y = OT^T @ Wo, fp16 partials -----
        ph2.close()
        pYp = st.enter_context(tc.tile_pool(name="pY", bufs=3, space="PSUM"))
        ysbp = st.enter_context(tc.tile_pool(name="ysbp", bufs=4))
        for t in range(NT):
            pY = pYp.tile([128, 1024], F32, name="pY")
            for e in range(2):
                for g in range(NP):
                    nc.tensor.matmul(
                        pY[:, e * 512:(e + 1) * 512],
                        ot_sb[:, g * S + t * 128:g * S + (t + 1) * 128],
                        wo_sb[:, g * D + e * 512:g * D + (e + 1) * 512],
                        start=(g == 0),
                        stop=(g == NP - 1),
                    )
            ysb = ysbp.tile([128, 1024], F16, name="ysb")
            if t % 2 == 0:
                nc.scalar.copy(out=ysb[:], in_=pY[:])
            else:
                nc.vector.tensor_copy(out=ysb[:], in_=pY[:])
            nc.sync.dma_start(out=Yp[t * 128:(t + 1) * 128, :], in_=ysb[:])

    nc.finalize()
    return nc


def make_in_maps(x, Wq, Wk, Wv, Wo):
    bf = ml_dtypes.bfloat16
    f = np.float32
    x = np.asarray(x, f)
    Wq, Wk, Wv, Wo = (np.asarray(a, f) for a in (Wq, Wk, Wv, Wo))
    in_maps = []
    xTs = [np.ascontiguousarray(x[b].T).astype(bf) for b in range(B)]

    def swz_qk(w):   # [D, DL] -> [128, NK*NP*128], block (k*NP+g)
        return np.ascontiguousarray(
            w.reshape(NK, 128, NP, 128).transpose(1, 0, 2, 3)
            .reshape(128, NK * NP * 128)).astype(bf)

    def swz_v(w):    # [D, DL] -> [128, NK*DL]
        return np.ascontiguousarray(
            w.reshape(NK, 128, DL).transpose(1, 0, 2)
            .reshape(128, NK * DL)).astype(bf)

    def swz_o(w):    # [DL, D] -> [128, NP*D]
        return np.ascontiguousarray(
            w.reshape(NP, 128, D).transpose(1, 0, 2)
            .reshape(128, NP * D)).astype(bf)

    for c in range(N_CORES):
        b, hb = divmod(c, N_CORES // B)
        cols = slice(hb * DL, (hb + 1) * DL)
        in_maps.append({
            "xT": xTs[b],
            "Wq": swz_qk(Wq[:, cols]),
            "Wk": swz_qk(Wk[:, cols]),
            "Wv": swz_v(Wv[:, cols]),
            "Wo": swz_o(Wo[cols, :] * f(1.0 / 32.0)),
        })
    return in_maps


def run(inputs, trace=False):
    nc = build_nc()
    in_maps = make_in_maps(**inputs)
    res = run_bass_kernel_spmd(nc, in_maps, list(range(N_CORES)), trace=trace)
    yps = [res.results[c]["Yp"] for c in range(N_CORES)]
    out = np.empty((B, S, D), np.float32)
    cpb = N_CORES // B
    for b in range(B):
        out[b] = np.sum([yps[b * cpb + i].astype(np.float32)
                         for i in range(cpb)], axis=0)
    return out, res


def kernel(**inputs):
    out, _ = run(inputs, trace=False)
    return out


# revision 18
# speedup vs baseline: 1.0311x; 1.0311x over previous
"""Multi-head attention TRN2 kernel, 8-core (batch x head-block) sharded.

Problem (hardcoded): x[2,2048,1024] f32, Wq/Wk/Wv[1024,1024], Wo[1024,1024],
16 heads, dh=64. Reference computes softmax(Q K^T)/sqrt(1024) @ V @ Wo with the
division AFTER softmax (folded here into Wo as a host-side 1/32 scale).

Sharding: core c handles batch b=c//4 and head block hb=c%4 (4 heads = 256 dims:
Wq/Wk/Wv column slice, Wo row slice). Each core emits a partial Y[2048,1024]
in fp16; host sums the 4 partials per batch in fp32.

v5: bf16 datapath, 64x128 row-tiled QK^T (two heads concurrently on PE tiles
T0/T8 since dh=64), single-op approx reciprocal, fp16 output partials, and a
software-pipelined schedule: only K + a quarter of Q are projected up front;
the attention exp stream (the ACT-engine bottleneck, ~1.1us per [128,1024]
tile) starts ~18us in and the remaining Q/V projections + deferred PV
matmuls fill the tensor engine's slack under it. This also keeps PE activity
high so the HAM clock gate stays at 2.4GHz (an idle phase transition
re-throttles it to 1.2GHz and phase 2 then never re-warms).
"""

from contextlib import ExitStack

import numpy as np
import ml_dtypes

import concourse.tile as tile
from concourse import bacc, mybir
from concourse.bass_utils import run_bass_kernel_spmd

N_CORES = 8
B = 2
S = 2048          # tokens per batch (= per core)
D = 1024          # model dim
DH = 64           # head dim
HPC = 4           # heads per core
NP = HPC // 2     # head pairs per core (pair p = heads 2p, 2p+1)
DL = HPC * DH     # 256 local output dims per core
NK = D // 128     # 8 k-strips for QKV contraction
NT = S // 128     # 16 token strips
VW = 128          # V block: cols 0:64 = V dims, cols 64:128 = ones (denoms)

F32 = mybir.dt.float32
F16 = mybir.dt.float16
BF16 = mybir.dt.bfloat16
EXP = mybir.ActivationFunctionType.Exp
MULT = mybir.AluOpType.mult


def build_nc():
    nc = bacc.Bacc("TRN2", target_bir_lowering=False, debug=False)
    # weights are host-swizzled into their SBUF layouts so each loads as one
    # contiguous full-line DMA
    xT = nc.declare_dram_parameter("xT", [D, S], BF16, isOutput=False)
    Wq = nc.declare_dram_parameter("Wq", [128, NK * NP * 128], BF16,
                                   isOutput=False)
    Wk = nc.declare_dram_parameter("Wk", [128, NK * NP * 128], BF16,
                                   isOutput=False)
    Wv = nc.declare_dram_parameter("Wv", [128, NK * DL], BF16, isOutput=False)
    Wo = nc.declare_dram_parameter("Wo", [128, NP * D], BF16, isOutput=False)
    Yp = nc.declare_dram_parameter("Yp", [S, D], F16, isOutput=True)

    with tile.TileContext(nc) as tc, ExitStack() as st:
        singles = st.enter_context(tc.tile_pool(name="singles", bufs=1))
        wq_sb = singles.tile([128, NK * NP * 128], BF16)
        wk_sb = singles.tile([128, NK * NP * 128], BF16)
        wv_sb = singles.tile([128, NK * DL], BF16)
        wo_sb = singles.tile([128, NP * D], BF16)
        # qt/kt: pair p block = cols [p*S, (p+1)*S); rows 0:64 = head 2p
        # dims, rows 64:128 = head 2p+1 dims (feeds PE tiles T0/T8).
        qt_sb = singles.tile([128, NP * S], BF16)
        kt_sb = singles.tile([128, NP * S], BF16)
        ot_sb = singles.tile([128, NP * S], BF16)
        vaug_sb = singles.tile([128, HPC * NT * VW], BF16)

        # ones columns of vaug (denominator rows of pO)
        for blk in range(HPC * NT):
            nc.gpsimd.memset(vaug_sb[:, blk * VW + DH:(blk + 1) * VW], 1.0)

        # phase-2 pools opened first: pools must close in LIFO order, and the
        # projection pools (pst) close mid-phase2 to hand their PSUM banks
        # to the pO accumulators; all phase-2 pools close before phase 3
        ph2 = ExitStack()
        pSp = ph2.enter_context(tc.tile_pool(name="pS", bufs=2, space="PSUM"))
        expp = ph2.enter_context(tc.tile_pool(name="expp", bufs=40))
        normp = ph2.enter_context(tc.tile_pool(name="normp", bufs=2))

        # ---- projection-phase pools (closed mid-phase2, before pO opens) --
        pst = ExitStack()
        xkp = pst.enter_context(tc.tile_pool(name="xkp", bufs=4))
        pp1 = pst.enter_context(tc.tile_pool(name="pp1", bufs=2, space="PSUM"))
        ppv = pst.enter_context(tc.tile_pool(name="ppv", bufs=2, space="PSUM"))

        xcs = []

        def load_chunk(c):
            xc = xkp.tile([128, NK * 512], BF16, name="xk")
            nc.sync.dma_start(
                out=xc[:].rearrange("p (k s) -> p k s", k=NK),
                in_=xT[:, c * 512:(c + 1) * 512]
                .rearrange("(k p) s -> p k s", p=128),
            )
            xcs.append(xc)

        load_chunk(0)
        nc.sync.dma_start(out=wk_sb[:], in_=Wk[:, :])
        nc.sync.dma_start(out=wq_sb[:], in_=Wq[:, :])
        load_chunk(1)
        load_chunk(2)
        load_chunk(3)
        nc.sync.dma_start(out=wv_sb[:], in_=Wv[:, :])
        nc.sync.dma_start(out=wo_sb[:], in_=Wo[:, :])

        lead_engines = (nc.scalar, nc.vector)
        eci = [0]

        def copy_any(dst, src, lead):
            # lead-in copies alternate ACT/DVE; during phase 2 ACT is the
            # exp bottleneck so everything goes to DVE
            eng = lead_engines[eci[0] % 2] if lead else nc.vector
            eci[0] += 1
            if eng is nc.scalar:
                eng.copy(out=dst, in_=src)
            else:
                eng.tensor_copy(out=dst, in_=src)

        def proj_qk(w_sb, o_sb, g, c, lead=False):
            ps = pp1.tile([128, 512], F32, name="ps_qk")
            for k in range(NK):
                cb = (k * NP + g) * 128
                nc.tensor.matmul(
                    ps[:],
                    w_sb[:, cb:cb + 128],
                    xcs[c][:, k * 512:(k + 1) * 512],
                    start=(k == 0),
                    stop=(k == NK - 1),
                )
            copy_any(o_sb[:, g * S + c * 512:g * S + (c + 1) * 512], ps[:],
                     lead)

        def proj_v(j, lead=False):
            c, t = divmod(j, 4)
            pv = ppv.tile([128, DL], F32, name="pv")
            for k in range(NK):
                nc.tensor.matmul(
                    pv[:],
                    xcs[c][:, k * 512 + t * 128:k * 512 + (t + 1) * 128],
                    wv_sb[:, k * DL:(k + 1) * DL],
                    start=(k == 0),
                    stop=(k == NK - 1),
                )
            for h in range(HPC):
                vb = (h * NT + j) * VW
                copy_any(vaug_sb[:, vb:vb + DH],
                         pv[:, h * DH:(h + 1) * DH], lead)

        # ---- lead-in: K pair 0 + the first Q quarter (pass (0,0) inputs) --
        proj_qk(wk_sb, kt_sb, 0, 0, lead=True)
        proj_qk(wk_sb, kt_sb, 0, 1, lead=True)
        proj_qk(wq_sb, qt_sb, 0, 0, lead=True)
        proj_qk(wk_sb, kt_sb, 0, 2, lead=True)
        proj_qk(wq_sb, qt_sb, 0, 1, lead=True)
        proj_qk(wk_sb, kt_sb, 0, 3, lead=True)

        # remaining projection work, paced into pass-(0,0) tensor slack;
        # (cost_us, emit_fn). K pair 1 and Qg1 c0/c1 are needed at pass
        # (1,0) start, V at the first deferred-PV drain, Q c2/c3 only by
        # the sh=1 passes.
        proj_units = [(1.7, lambda c=c: proj_qk(wk_sb, kt_sb, 1, c))
                      for c in range(4)]
        proj_units += [(1.7, lambda g=1, c=0: proj_qk(wq_sb, qt_sb, g, c)),
                       (1.7, lambda g=1, c=1: proj_qk(wq_sb, qt_sb, g, c))]
        proj_units += [(0.85, lambda j=j: proj_v(j)) for j in range(NT)]
        proj_units += [(1.7, lambda g=g, c=c: proj_qk(wq_sb, qt_sb, g, c))
                       for g, c in ((0, 2), (0, 3), (1, 2), (1, 3))]

        # ---- phase 2: attention ------------------------------------------
        pOp_box = [None]

        pO_tiles = {}
        pv_fifo = []

        def emit_qk_exp(p, sh, j, half):
            r0 = half * DH
            pS = pSp.tile([128, 1024], F32, name="pS")
            for sc in range(2):
                nc.tensor.matmul(
                    pS[:, sc * 512:(sc + 1) * 512],
                    kt_sb[r0:r0 + DH,
                          p * S + j * 128:p * S + (j + 1) * 128],
                    qt_sb[r0:r0 + DH,
                          p * S + sh * 1024 + sc * 512:
                          p * S + sh * 1024 + (sc + 1) * 512],
                )
            expst = expp.tile([128, 1024], BF16, name="expst")
            nc.scalar.activation(expst[:], pS[:], EXP)
            pv_fifo.append((expst, p, sh, j, half))

        def drain_pv(n):
            for _ in range(min(n, len(pv_fifo))):
                expst, p, sh, j, half = pv_fifo.pop(0)
                key = (p, sh, half)
                if key not in pO_tiles:
                    pO_tiles[key] = pOp_box[0].tile([VW, 1024], F32,
                                                    name="pO")
                pO = pO_tiles[key]
                h = 2 * p + half
                vb = (h * NT + j) * VW
                for sc in range(2):
                    nc.tensor.matmul(
                        pO[:, sc * 512:(sc + 1) * 512],
                        vaug_sb[:, vb:vb + VW],
                        expst[:, sc * 512:(sc + 1) * 512],
                        start=(j == 0),
                        stop=(j == NT - 1),
                        skip_group_check=True,
                    )
                if j == NT - 1:
                    emit_norm(p, sh, half)

        def emit_norm(p, sh, half):
            # pO rows 64:128 all hold the softmax denominator (vaug cols
            # 64:128 are ones): one approx reciprocal (custom-DVE ops drop
            # the input partition offset, so run it over all 128 partitions
            # and use rows 64:128), then scale the V rows.
            pO = pO_tiles.pop((p, sh, half))
            r = half * DH
            rb = normp.tile([128, 1024], F32, name="rb")
            nc.vector.reciprocal_approx_fast(rb[:], pO[:])
            nc.vector.tensor_tensor(
                out=ot_sb[r:r + DH,
                          p * S + sh * 1024:p * S + (sh + 1) * 1024],
                in0=pO[0:DH, :],
                in1=rb[DH:128, :],
                op=MULT,
            )

        # pass (0,0): QK+exp only (PV deferred until V is projected); the
        # remaining projections are paced into the slack by a credit budget
        # (~0.9us of PE slack per strip-half at the ~1.1us exp cadence).
        credits = 0.0
        for j in range(NT):
            for half in range(2):
                emit_qk_exp(0, 0, j, half)
                credits += 1.0
                while proj_units and credits >= proj_units[0][0]:
                    cost, fn = proj_units.pop(0)
                    credits -= cost
                    fn()
        # spill any leftover projection work, then free its PSUM pools so
        # the pO accumulators can take those banks
        for _, fn in proj_units:
            fn()
        pst.close()
        pOp_box[0] = ph2.enter_context(
            tc.tile_pool(name="pO", bufs=2, space="PSUM"))

        for pi, (p, sh) in enumerate(((1, 0), (0, 1), (1, 1))):
            for j in range(NT):
                for half in range(2):
                    emit_qk_exp(p, sh, j, half)
                    # skip drains for the first strips of a pass: the first
                    # PV of a pass waits for a pO slot (freed by the prev
                    # pass's norm) and would head-of-line-block later QKs;
                    # drain harder late in the last pass to shrink the
                    # post-exp tail
                    if j >= 3:
                        drain_pv(3 if (pi == 2 and j >= 12) else 2)
        drain_pv(len(pv_fifo))

        # ---- phase 3: output projection Y = OT^T @ Wo, fp16 partials -----
        ph2.close()
        pYp = st.enter_context(tc.tile_pool(name="pY", bufs=3, space="PSUM"))
        ysbp = st.enter_context(tc.tile_pool(name="ysbp", bufs=4))
        for t in range(NT):
            pY = pYp.tile([128, 1024], F32, name="pY")
            for e in range(2):
                for g in range(NP):
                    nc.tensor.matmul(
                        pY[:, e * 512:(e + 1) * 512],
                        ot_sb[:, g * S + t * 128:g * S + (t + 1) * 128],
                        wo_sb[:, g * D + e * 512:g * D + (e + 1) * 512],
                        start=(g == 0),
                        stop=(g == NP - 1),
                    )
            ysb = ysbp.tile([128, 1024], F16, name="ysb")
            if t % 2 == 0:
                nc.scalar.copy(out=ysb[:], in_=pY[:])
            else:
                nc.vector.tensor_copy(out=ysb[:], in_=pY[:])
            nc.sync.dma_start(out=Yp[t * 128:(t + 1) * 128, :], in_=ysb[:])

    nc.finalize()
    return nc


def make_in_maps(x, Wq, Wk, Wv, Wo):
    bf = ml_dtypes.bfloat16
    f = np.float32
    x = np.asarray(x, f)
    Wq, Wk, Wv, Wo = (np.asarray(a, f) for a in (Wq, Wk, Wv, Wo))
    in_maps = []
    xTs = [np.ascontiguousarray(x[b].T).astype(bf) for b in range(B)]

    def swz_qk(w):   # [D, DL] -> [128, NK*NP*128], block (k*NP+g)
        return np.ascontiguousarray(
            w.reshape(NK, 128, NP, 128).transpose(1, 0, 2, 3)
            .reshape(128, NK * NP * 128)).astype(bf)

    def swz_v(w):    # [D, DL] -> [128, NK*DL]
        return np.ascontiguousarray(
            w.reshape(NK, 128, DL).transpose(1, 0, 2)
            .reshape(128, NK * DL)).astype(bf)

    def swz_o(w):    # [DL, D] -> [128, NP*D]
        return np.ascontiguousarray(
            w.reshape(NP, 128, D).transpose(1, 0, 2)
            .reshape(128, NP * D)).astype(bf)

    for c in range(N_CORES):
        b, hb = divmod(c, N_CORES // B)
        cols = slice(hb * DL, (hb + 1) * DL)
        in_maps.append({
            "xT": xTs[b],
            "Wq": swz_qk(Wq[:, cols]),
            "Wk": swz_qk(Wk[:, cols]),
            "Wv": swz_v(Wv[:, cols]),
            "Wo": swz_o(Wo[cols, :] * f(1.0 / 32.0)),
        })
    return in_maps


def run(inputs, trace=False):
    nc = build_nc()
    in_maps = make_in_maps(**inputs)
    res = run_bass_kernel_spmd(nc, in_maps, list(range(N_CORES)), trace=trace)
    yps = [res.results[c]["Yp"] for c in range(N_CORES)]
    out = np.empty((B, S, D), np.float32)
    cpb = N_CORES // B
    for b in range(B):
        out[b] = np.sum([yps[b * cpb + i].astype(np.float32)
                         for i in range(cpb)], axis=0)
    return out, res


def kernel(**inputs):
    out, _ = run(inputs, trace=False)
    return out


# revision 20
# speedup vs baseline: 1.0459x; 1.0143x over previous
"""Multi-head attention TRN2 kernel, 8-core (batch x head-block) sharded.

Problem (hardcoded): x[2,2048,1024] f32, Wq/Wk/Wv[1024,1024], Wo[1024,1024],
16 heads, dh=64. Reference computes softmax(Q K^T)/sqrt(1024) @ V @ Wo with the
division AFTER softmax (folded here into Wo as a host-side 1/32 scale).

Sharding: core c handles batch b=c//4 and head block hb=c%4 (4 heads = 256 dims:
Wq/Wk/Wv column slice, Wo row slice). Each core emits a partial Y[2048,1024]
in fp16; host sums the 4 partials per batch in fp32.

v5: bf16 datapath, 64x128 row-tiled QK^T (two heads concurrently on PE tiles
T0/T8 since dh=64), single-op approx reciprocal, fp16 output partials, and a
software-pipelined schedule: only K + a quarter of Q are projected up front;
the attention exp stream (the ACT-engine bottleneck, ~1.1us per [128,1024]
tile) starts ~18us in and the remaining Q/V projections + deferred PV
matmuls fill the tensor engine's slack under it. This also keeps PE activity
high so the HAM clock gate stays at 2.4GHz (an idle phase transition
re-throttles it to 1.2GHz and phase 2 then never re-warms).
"""

from contextlib import ExitStack

import numpy as np
import ml_dtypes

import concourse.tile as tile
from concourse import bacc, mybir
from concourse.bass_utils import run_bass_kernel_spmd

N_CORES = 8
B = 2
S = 2048          # tokens per batch (= per core)
D = 1024          # model dim
DH = 64           # head dim
HPC = 4           # heads per core
NP = HPC // 2     # head pairs per core (pair p = heads 2p, 2p+1)
DL = HPC * DH     # 256 local output dims per core
NK = D // 128     # 8 k-strips for QKV contraction
NT = S // 128     # 16 token strips
VW = 128          # V block: cols 0:64 = V dims, cols 64:128 = ones (denoms)

F32 = mybir.dt.float32
F16 = mybir.dt.float16
BF16 = mybir.dt.bfloat16
EXP = mybir.ActivationFunctionType.Exp
MULT = mybir.AluOpType.mult


def build_nc():
    nc = bacc.Bacc("TRN2", target_bir_lowering=False, debug=False)
    # weights are host-swizzled into their SBUF layouts so each loads as one
    # contiguous full-line DMA
    xT = nc.declare_dram_parameter("xT", [D, S], BF16, isOutput=False)
    Wq = nc.declare_dram_parameter("Wq", [128, NK * NP * 128], BF16,
                                   isOutput=False)
    Wk = nc.declare_dram_parameter("Wk", [128, NK * NP * 128], BF16,
                                   isOutput=False)
    Wv = nc.declare_dram_parameter("Wv", [128, NK * DL], BF16, isOutput=False)
    Wo = nc.declare_dram_parameter("Wo", [128, NP * D], BF16, isOutput=False)
    Yp = nc.declare_dram_parameter("Yp", [S, D], F16, isOutput=True)

    with tile.TileContext(nc) as tc, ExitStack() as st:
        singles = st.enter_context(tc.tile_pool(name="singles", bufs=1))
        wq_sb = singles.tile([128, NK * NP * 128], BF16)
        wk_sb = singles.tile([128, NK * NP * 128], BF16)
        wv_sb = singles.tile([128, NK * DL], BF16)
        wo_sb = singles.tile([128, NP * D], BF16)
        # qt/kt: pair p block = cols [p*S, (p+1)*S); rows 0:64 = head 2p
        # dims, rows 64:128 = head 2p+1 dims (feeds PE tiles T0/T8).
        qt_sb = singles.tile([128, NP * S], BF16)
        kt_sb = singles.tile([128, NP * S], BF16)
        ot_sb = singles.tile([128, NP * S], BF16)
        vaug_sb = singles.tile([128, HPC * NT * VW], BF16)

        # ones columns of vaug (denominator rows of pO)
        for blk in range(HPC * NT):
            nc.gpsimd.memset(vaug_sb[:, blk * VW + DH:(blk + 1) * VW], 1.0)

        # phase-2 pools opened first: pools must close in LIFO order and the
        # projection pools (pst) close mid-phase2 to hand their PSUM banks
        # to the pO accumulators. Phase 3 reuses the pS slots ([128,1024]
        # PSUM) directly, so no pool transition separates it from the last
        # pass and the PE never idles long enough for HAM to re-throttle.
        pSp = st.enter_context(tc.tile_pool(name="pS", bufs=2, space="PSUM"))
        expp = st.enter_context(tc.tile_pool(name="expp", bufs=34))
        normp = st.enter_context(tc.tile_pool(name="normp", bufs=2))
        ysbp = st.enter_context(tc.tile_pool(name="ysbp", bufs=4))

        # ---- projection-phase pools (closed mid-phase2, before pO opens) --
        pst = ExitStack()
        xkp = pst.enter_context(tc.tile_pool(name="xkp", bufs=4))
        pp1 = pst.enter_context(tc.tile_pool(name="pp1", bufs=2, space="PSUM"))
        ppv = pst.enter_context(tc.tile_pool(name="ppv", bufs=2, space="PSUM"))

        xcs = []

        def load_chunk(c):
            xc = xkp.tile([128, NK * 512], BF16, name="xk")
            nc.sync.dma_start(
                out=xc[:].rearrange("p (k s) -> p k s", k=NK),
                in_=xT[:, c * 512:(c + 1) * 512]
                .rearrange("(k p) s -> p k s", p=128),
            )
            xcs.append(xc)

        load_chunk(0)
        nc.sync.dma_start(out=wk_sb[:], in_=Wk[:, :])
        nc.sync.dma_start(out=wq_sb[:], in_=Wq[:, :])
        load_chunk(1)
        load_chunk(2)
        load_chunk(3)
        nc.sync.dma_start(out=wv_sb[:], in_=Wv[:, :])
        nc.sync.dma_start(out=wo_sb[:], in_=Wo[:, :])

        lead_engines = (nc.scalar, nc.vector)
        eci = [0]

        def copy_any(dst, src, lead):
            # lead-in copies alternate ACT/DVE; during phase 2 ACT is the
            # exp bottleneck so everything goes to DVE
            eng = lead_engines[eci[0] % 2] if lead else nc.vector
            eci[0] += 1
            if eng is nc.scalar:
                eng.copy(out=dst, in_=src)
            else:
                eng.tensor_copy(out=dst, in_=src)

        def proj_qk(w_sb, o_sb, g, c, lead=False):
            ps = pp1.tile([128, 512], F32, name="ps_qk")
            for k in range(NK):
                cb = (k * NP + g) * 128
                nc.tensor.matmul(
                    ps[:],
                    w_sb[:, cb:cb + 128],
                    xcs[c][:, k * 512:(k + 1) * 512],
                    start=(k == 0),
                    stop=(k == NK - 1),
                )
            copy_any(o_sb[:, g * S + c * 512:g * S + (c + 1) * 512], ps[:],
                     lead)

        def proj_v(j, lead=False):
            c, t = divmod(j, 4)
            pv = ppv.tile([128, DL], F32, name="pv")
            for k in range(NK):
                nc.tensor.matmul(
                    pv[:],
                    xcs[c][:, k * 512 + t * 128:k * 512 + (t + 1) * 128],
                    wv_sb[:, k * DL:(k + 1) * DL],
                    start=(k == 0),
                    stop=(k == NK - 1),
                )
            for h in range(HPC):
                vb = (h * NT + j) * VW
                copy_any(vaug_sb[:, vb:vb + DH],
                         pv[:, h * DH:(h + 1) * DH], lead)

        # ---- lead-in: K pair 0 + the first Q quarter (pass (0,0) inputs) --
        proj_qk(wk_sb, kt_sb, 0, 0, lead=True)
        proj_qk(wk_sb, kt_sb, 0, 1, lead=True)
        proj_qk(wq_sb, qt_sb, 0, 0, lead=True)
        proj_qk(wk_sb, kt_sb, 0, 2, lead=True)
        proj_qk(wq_sb, qt_sb, 0, 1, lead=True)
        proj_qk(wk_sb, kt_sb, 0, 3, lead=True)

        # remaining projection work, paced into pass-(0,0) tensor slack;
        # (cost_us, emit_fn). K pair 1 and Qg1 c0/c1 are needed at pass
        # (1,0) start, V at the first deferred-PV drain, Q c2/c3 only by
        # the sh=1 passes.
        proj_units = [(1.7, lambda c=c: proj_qk(wk_sb, kt_sb, 1, c))
                      for c in range(4)]
        proj_units += [(1.7, lambda g=1, c=0: proj_qk(wq_sb, qt_sb, g, c)),
                       (1.7, lambda g=1, c=1: proj_qk(wq_sb, qt_sb, g, c))]
        proj_units += [(0.85, lambda j=j: proj_v(j)) for j in range(NT)]
        proj_units += [(1.7, lambda g=g, c=c: proj_qk(wq_sb, qt_sb, g, c))
                       for g, c in ((0, 2), (0, 3), (1, 2), (1, 3))]

        # ---- phase 2: attention ------------------------------------------
        pOp_box = [None]

        pO_tiles = {}
        pv_fifo = []

        def emit_qk_exp(p, sh, j, half):
            r0 = half * DH
            pS = pSp.tile([128, 1024], F32, name="pS")
            for sc in range(2):
                nc.tensor.matmul(
                    pS[:, sc * 512:(sc + 1) * 512],
                    kt_sb[r0:r0 + DH,
                          p * S + j * 128:p * S + (j + 1) * 128],
                    qt_sb[r0:r0 + DH,
                          p * S + sh * 1024 + sc * 512:
                          p * S + sh * 1024 + (sc + 1) * 512],
                )
            expst = expp.tile([128, 1024], BF16, name="expst")
            nc.scalar.activation(expst[:], pS[:], EXP)
            pv_fifo.append((expst, p, sh, j, half))

        def drain_pv(n):
            for _ in range(min(n, len(pv_fifo))):
                expst, p, sh, j, half = pv_fifo.pop(0)
                key = (p, sh, half)
                if key not in pO_tiles:
                    pO_tiles[key] = pOp_box[0].tile([VW, 1024], F32,
                                                    name="pO")
                pO = pO_tiles[key]
                h = 2 * p + half
                vb = (h * NT + j) * VW
                for sc in range(2):
                    nc.tensor.matmul(
                        pO[:, sc * 512:(sc + 1) * 512],
                        vaug_sb[:, vb:vb + VW],
                        expst[:, sc * 512:(sc + 1) * 512],
                        start=(j == 0),
                        stop=(j == NT - 1),
                        skip_group_check=True,
                    )
                if j == NT - 1:
                    emit_norm(p, sh, half)

        def emit_norm(p, sh, half):
            # pO rows 64:128 all hold the softmax denominator (vaug cols
            # 64:128 are ones): one approx reciprocal (custom-DVE ops drop
            # the input partition offset, so run it over all 128 partitions
            # and use rows 64:128), then scale the V rows.
            pO = pO_tiles.pop((p, sh, half))
            r = half * DH
            rb = normp.tile([128, 1024], F32, name="rb")
            nc.vector.reciprocal_approx_fast(rb[:], pO[:])
            nc.vector.tensor_tensor(
                out=ot_sb[r:r + DH,
                          p * S + sh * 1024:p * S + (sh + 1) * 1024],
                in0=pO[0:DH, :],
                in1=rb[DH:128, :],
                op=MULT,
            )

        # pass (0,0): QK+exp only (PV deferred until V is projected); the
        # remaining projections are paced into the slack by a credit budget
        # (~0.9us of PE slack per strip-half at the ~1.1us exp cadence).
        credits = 0.0
        for j in range(NT):
            for half in range(2):
                emit_qk_exp(0, 0, j, half)
                credits += 1.0
                while proj_units and credits >= proj_units[0][0]:
                    cost, fn = proj_units.pop(0)
                    credits -= cost
                    fn()
        # spill any leftover projection work, then free its PSUM pools so
        # the pO accumulators can take those banks
        for _, fn in proj_units:
            fn()
        pst.close()
        pOp_box[0] = st.enter_context(
            tc.tile_pool(name="pO", bufs=2, space="PSUM"))

        for pi, (p, sh) in enumerate(((1, 0), (0, 1), (1, 1))):
            for j in range(NT):
                for half in range(2):
                    emit_qk_exp(p, sh, j, half)
                    # skip drains for the first strips of a pass: the first
                    # PV of a pass waits for a pO slot (freed by the prev
                    # pass's norm) and would head-of-line-block later QKs;
                    # drain harder late in the last pass to shrink the
                    # post-exp tail
                    if j >= 3:
                        drain_pv(3 if (pi == 2 and j >= 12) else 2)
        drain_pv(len(pv_fifo))

        # ---- phase 3: output projection Y = OT^T @ Wo, fp16 partials -----
        for t in range(NT):
            pY = pSp.tile([128, 1024], F32, name="pS")
            for e in range(2):
                for g in range(NP):
                    nc.tensor.matmul(
                        pY[:, e * 512:(e + 1) * 512],
                        ot_sb[:, g * S + t * 128:g * S + (t + 1) * 128],
                        wo_sb[:, g * D + e * 512:g * D + (e + 1) * 512],
                        start=(g == 0),
                        stop=(g == NP - 1),
                    )
            ysb = ysbp.tile([128, 1024], F16, name="ysb")
            if t % 2 == 0:
                nc.scalar.copy(out=ysb[:], in_=pY[:])
            else:
                nc.vector.tensor_copy(out=ysb[:], in_=pY[:])
            nc.sync.dma_start(out=Yp[t * 128:(t + 1) * 128, :], in_=ysb[:])

    nc.finalize()
    return nc


def make_in_maps(x, Wq, Wk, Wv, Wo):
    bf = ml_dtypes.bfloat16
    f = np.float32
    x = np.asarray(x, f)
    Wq, Wk, Wv, Wo = (np.asarray(a, f) for a in (Wq, Wk, Wv, Wo))
    in_maps = []
    xTs = [np.ascontiguousarray(x[b].T).astype(bf) for b in range(B)]

    def swz_qk(w):   # [D, DL] -> [128, NK*NP*128], block (k*NP+g)
        return np.ascontiguousarray(
            w.reshape(NK, 128, NP, 128).transpose(1, 0, 2, 3)
            .reshape(128, NK * NP * 128)).astype(bf)

    def swz_v(w):    # [D, DL] -> [128, NK*DL]
        return np.ascontiguousarray(
            w.reshape(NK, 128, DL).transpose(1, 0, 2)
            .reshape(128, NK * DL)).astype(bf)

    def swz_o(w):    # [DL, D] -> [128, NP*D]
        return np.ascontiguousarray(
            w.reshape(NP, 128, D).transpose(1, 0, 2)
            .reshape(128, NP * D)).astype(bf)

    for c in range(N_CORES):
        b, hb = divmod(c, N_CORES // B)
        cols = slice(hb * DL, (hb + 1) * DL)
        in_maps.append({
            "xT": xTs[b],
            "Wq": swz_qk(Wq[:, cols]),
            "Wk": swz_qk(Wk[:, cols]),
            "Wv": swz_v(Wv[:, cols]),
            "Wo": swz_o(Wo[cols, :] * f(1.0 / 32.0)),
        })
    return in_maps


def run(inputs, trace=False):
    nc = build_nc()
    in_maps = make_in_maps(**inputs)
    res = run_bass_kernel_spmd(nc, in_maps, list(range(N_CORES)), trace=trace)
    yps = [res.results[c]["Yp"] for c in range(N_CORES)]
    out = np.empty((B, S, D), np.float32)
    cpb = N_CORES // B
    for b in range(B):
        out[b] = np.sum([yps[b * cpb + i].astype(np.float32)
                         for i in range(cpb)], axis=0)
    return out, res


def kernel(**inputs):
    out, _ = run(inputs, trace=False)
    return out
